# revision 14
# baseline (speedup 1.0000x reference)
"""AdaptiveClusteringAttention TRN2 kernel.

Data-parallel over batch: b=8 rows -> 8 NeuronCores, one row per core,
weights replicated. No collectives.

Per-core math (n=4096 tokens, d=1024, C=256 clusters, H=16 heads, dh=64):
  xc[c,:]   = sum_{t: cluster[t]=c} x[t,:]          (onehot matmul)
  cnt[c]    = |{t: cluster[t]=c}|
  xm[c,:]   = xc[c,:] / max(cnt[c], .5)
  kc        = xm @ w_k ; vc = xm @ w_v              (segmean commutes with proj)
  qh        = x @ w_q
  s[t,c]    = qh_h[t] . kc_h[c] / 8
  attn      = softmax(s + log cnt)                  (count-weighted softmax)
  out       = attn @ vc ; y = out @ w_proj + b_proj

Layouts: everything d-major ("transposed") so x^T is the only transpose,
done via f32->bf16 cast-DMA + XBAR DMA-transpose. Matmuls are bf16 on the
q/score/attn path and float32r elsewhere. exp is fused into the scores
PSUM eviction (scale=1/8, bias=log-counts per partition). sum-exp comes
free from a ones column appended to vc; 1/sumexp is broadcast across
partitions with a K=1 matmul.
"""

import os
import sys

import numpy as np

for _p in ("/opt/trn_rl_repo", os.path.expanduser("~/.axon_site/_ro/trn_rl_repo")):
    if os.path.isdir(_p) and _p not in sys.path:
        sys.path.append(_p)

import concourse.bass as bass  # noqa: E402
import concourse.mybir as mybir  # noqa: E402
import concourse.tile as tile  # noqa: E402
from concourse import bacc  # noqa: E402
from concourse.masks import make_identity  # noqa: E402

FP32 = mybir.dt.float32
BF16 = mybir.dt.bfloat16
I32 = mybir.dt.int32

N, D, C, H, DH, P = 4096, 1024, 256, 16, 64, 128
NJ = N // P          # 32 token row-tiles
NK = D // P          # 8 contraction chunks
TCH = 512            # token chunk for the attention phase
NCH = N // TCH       # 8 chunks
NMT = TCH // P       # 4 token subtiles per chunk

TRACE = False
LAST_RESULTS = None


def build_nc():
    nc = bacc.Bacc("TRN2", target_bir_lowering=False, debug=False)

    x_d = nc.dram_tensor("x", [N, D], FP32, kind="ExternalInput").ap()
    cl_d = nc.dram_tensor("cluster", [N], I32, kind="ExternalInput").ap()
    wq_d = nc.dram_tensor("w_q", [D, D], FP32, kind="ExternalInput").ap()
    wk_d = nc.dram_tensor("w_k", [D, D], FP32, kind="ExternalInput").ap()
    wv_d = nc.dram_tensor("w_v", [D, D], FP32, kind="ExternalInput").ap()
    wp_d = nc.dram_tensor("w_proj", [D, D], FP32, kind="ExternalInput").ap()
    bp_d = nc.dram_tensor("b_proj", [1, D], FP32, kind="ExternalInput").ap()
    out_d = nc.dram_tensor("out", [N, D], FP32, kind="ExternalOutput").ap()

    with tile.TileContext(nc) as tc:
        with (
            tc.tile_pool(name="dram", bufs=1, space="DRAM") as dram,
            tc.tile_pool(name="wts", bufs=1) as wts,
        ):
            xbf_d = dram.tile([N, D], BF16)

            # ---- constants ----
            iota_i = wts.tile([P, C], I32, tag="iota_i")
            nc.gpsimd.iota(iota_i[:], pattern=[[1, C]], base=0, channel_multiplier=0)
            iota_b = wts.tile([P, C], BF16, tag="iota_b")
            nc.vector.tensor_copy(iota_b[:], iota_i[:])
            ident = wts.tile([32, 32], BF16, tag="ident")
            make_identity(nc, ident[:])
            ones_col = wts.tile([P, 1], BF16, tag="ones_col")
            nc.vector.memset(ones_col[:], 1.0)
            ones_row = wts.tile([1, 64], BF16, tag="ones_row")
            nc.vector.memset(ones_row[:], 1.0)

            bp_sb = wts.tile([1, D], FP32, tag="bp_sb")
            nc.sync.dma_start(out=bp_sb[:], in_=bp_d)
            b_bc = wts.tile([P, D], FP32, tag="b_bc")
            nc.gpsimd.partition_broadcast(b_bc[:], bp_sb[:])

            cl_i = wts.tile([NJ, P], I32, tag="cl_i")
            nc.sync.dma_start(out=cl_i[:], in_=cl_d.rearrange("(a b) -> a b", b=P))
            cl_b = wts.tile([NJ, P], BF16, tag="cl_b")
            nc.vector.tensor_copy(cl_b[:], cl_i[:])
            clusT = wts.tile([P, NJ], FP32, tag="clusT")
            with tc.tile_pool(name="psct", bufs=1, space="PSUM") as psct:
                ct_ps = psct.tile([P, NJ], BF16, tag="ct")
                nc.tensor.transpose(ct_ps[:], cl_b[:], ident[:])
                nc.vector.tensor_copy(clusT[:], ct_ps[:])

            # weights (bf16 via cast-DMA)
            wq_sb, wp_sb = [], []
            for k in range(NK):
                t = wts.tile([P, D], BF16, tag=f"wq{k}", name=f"wq{k}")
                nc.gpsimd.dma_start(out=t[:], in_=wq_d[k * P:(k + 1) * P, :])
                wq_sb.append(t)
                t = wts.tile([P, D], BF16, tag=f"wp{k}", name=f"wp{k}")
                nc.gpsimd.dma_start(out=t[:], in_=wp_d[k * P:(k + 1) * P, :])
                wp_sb.append(t)

            # ---- phase A: stream x (f32 DMA + DVE cast), onehot + counts ----
            xcm = [wts.tile([P, C], BF16, tag=f"xcm{m}", name=f"xcm{m}")
                   for m in range(NK)]
            cnt_sb = wts.tile([1, C], FP32, tag="cnt_sb")
            logc = wts.tile([P, 2], FP32, tag="logc")
            with (
                tc.tile_pool(name="psA", bufs=1, space="PSUM") as psA,
                tc.tile_pool(name="psAm", bufs=2, space="PSUM") as psAm,
                tc.tile_pool(name="xin", bufs=1) as xin,
                tc.tile_pool(name="xf32", bufs=6) as xf32,
                tc.tile_pool(name="ohp", bufs=1) as ohp,
            ):
                pcnt = psA.tile([1, C], FP32, tag="cnt")
                pcT = [psA.tile([P, 1], FP32, tag=f"cntT{i}", name=f"pcT{i}")
                       for i in range(2)]
                xall, ohall = [], []
                for j in range(NJ):
                    xf = xf32.tile([P, D], FP32, tag="xf")
                    nc.sync.dma_start(out=xf[:], in_=x_d[j * P:(j + 1) * P, :])
                    xj = xin.tile([P, D], BF16, tag=f"xj{j}", name=f"xj{j}")
                    nc.vector.tensor_copy(xj[:], xf[:])
                    nc.scalar.dma_start(out=xbf_d[j * P:(j + 1) * P, :], in_=xj[:])
                    oh = ohp.tile([P, C], BF16, tag=f"oh{j}", name=f"oh{j}")
                    nc.vector.tensor_scalar(
                        oh[:], iota_b[:], clusT[:, j:j + 1], None,
                        mybir.AluOpType.is_equal,
                    )
                    st, sp = (j == 0), (j == NJ - 1)
                    nc.tensor.matmul(pcnt[:], ones_col[:], oh[:], start=st, stop=sp)
                    for mc in range(2):
                        nc.tensor.matmul(
                            pcT[mc][:], oh[:, mc * P:(mc + 1) * P],
                            ones_col[:], start=st, stop=sp,
                        )
                    xall.append(xj)
                    ohall.append(oh)

                # counts -> inv (row + bcast), log-counts (column layout)
                nc.scalar.copy(cnt_sb[:], pcnt[:])
                cm_row = wts.tile([1, C], FP32, tag="cm_row")
                nc.vector.tensor_scalar_max(cm_row[:], cnt_sb[:], 0.5)
                inv_row = wts.tile([1, C], FP32, tag="inv_row")
                nc.vector.reciprocal(inv_row[:], cm_row[:])
                inv_bc = wts.tile([P, C], FP32, tag="inv_bc")
                nc.gpsimd.partition_broadcast(inv_bc[:], inv_row[:])

                cnt_col = wts.tile([P, 2], FP32, tag="cnt_col")
                for mc in range(2):
                    nc.scalar.copy(cnt_col[:, mc:mc + 1], pcT[mc][:])
                cm_col = wts.tile([P, 2], FP32, tag="cm_col")
                nc.vector.tensor_scalar_max(cm_col[:], cnt_col[:], 0.5)
                lg_col = wts.tile([P, 2], FP32, tag="lg_col")
                nc.scalar.activation(lg_col[:], cm_col[:],
                                     mybir.ActivationFunctionType.Ln)
                msk = wts.tile([P, 2], FP32, tag="msk")
                nc.vector.tensor_scalar(
                    msk[:], cnt_col[:], 0.5, 30.0,
                    mybir.AluOpType.is_lt, mybir.AluOpType.mult,
                )
                nc.vector.tensor_sub(logc[:], lg_col[:], msk[:])

                # xm^T = xc^T * inv  (d-major cluster means)
                for m in range(NK):
                    pxc = psAm.tile([P, C], FP32, tag="pxc")
                    for j in range(NJ):
                        nc.tensor.matmul(
                            pxc[:], xall[j][:, m * P:(m + 1) * P], ohall[j][:],
                            start=(j == 0), stop=(j == NJ - 1),
                        )
                    nc.vector.tensor_mul(xcm[m][:], pxc[:], inv_bc[:])

            # ---- phase B: kc^T and vc (with ones column) ----
            kc_sb = [wts.tile([P, C], BF16, tag=f"kc{m}", name=f"kc{m}")
                     for m in range(NK)]
            vca = [wts.tile([P, 16 * 65], BF16, tag=f"vca{i}", name=f"vca{i}")
                   for i in range(2)]
            for i in range(2):
                va = vca[i].rearrange("p (h e) -> p h e", e=65)
                nc.vector.memset(va[:, :, 64:65], 1.0)
            with (
                tc.tile_pool(name="psBk", bufs=2, space="PSUM") as psBk,
                tc.tile_pool(name="psBv", bufs=4, space="PSUM") as psBv,
                tc.tile_pool(name="wkv", bufs=1) as wkv,
            ):
                wk_sb, wv_sb = [], []
                for k in range(NK):
                    t = wkv.tile([P, D], BF16, tag=f"wk{k}", name=f"wk{k}")
                    nc.gpsimd.dma_start(out=t[:], in_=wk_d[k * P:(k + 1) * P, :])
                    wk_sb.append(t)
                    t = wkv.tile([P, D], BF16, tag=f"wv{k}", name=f"wv{k}")
                    nc.gpsimd.dma_start(out=t[:], in_=wv_d[k * P:(k + 1) * P, :])
                    wv_sb.append(t)
                for m in range(NK):
                    pk = psBk.tile([P, C], FP32, tag="pk")
                    for k in range(NK):
                        nc.tensor.matmul(
                            pk[:], wk_sb[k][:, m * P:(m + 1) * P],
                            xcm[k][:], start=(k == 0), stop=(k == NK - 1),
                        )
                    nc.vector.tensor_copy(kc_sb[m][:], pk[:])
                for mc in range(2):
                    va = vca[mc].rearrange("p (h e) -> p h e", e=65)
                    for nn in range(2):
                        pv = psBv.tile([P, 512], FP32, tag="pv")
                        for k in range(NK):
                            nc.tensor.matmul(
                                pv[:], xcm[k][:, mc * P:(mc + 1) * P],
                                wv_sb[k][:, nn * 512:(nn + 1) * 512],
                                start=(k == 0), stop=(k == NK - 1),
                            )
                        nc.vector.tensor_copy(
                            va[:, nn * 8:(nn + 1) * 8, 0:64],
                            pv.rearrange("p (h e) -> p h e", e=64),
                        )

            # ---- phase C/D: per token-chunk attention + output proj ----
            with (
                tc.tile_pool(name="xtp", bufs=2) as xtp,
                tc.tile_pool(name="qhp", bufs=2) as qhp,
                tc.tile_pool(name="expp", bufs=4) as expp,
                tc.tile_pool(name="sep", bufs=2) as sep,
                tc.tile_pool(name="otp", bufs=2) as otp,
                tc.tile_pool(name="finp", bufs=4) as finp,
                tc.tile_pool(name="psq", bufs=1, space="PSUM") as psq,
                tc.tile_pool(name="pss", bufs=1, space="PSUM") as pss,
                tc.tile_pool(name="psav", bufs=2, space="PSUM") as psav,
                tc.tile_pool(name="psf", bufs=2, space="PSUM") as psf,
            ):
                for ch in range(NCH):
                    t0 = ch * TCH
                    xT = []
                    for k in range(NK):
                        t = xtp.tile([P, TCH], BF16, tag=f"xt{k}", name=f"xt{k}")
                        nc.sync.dma_start_transpose(
                            out=t[:], in_=xbf_d[t0:t0 + TCH, k * P:(k + 1) * P]
                        )
                        xT.append(t)
                    qh = []
                    for m in range(NK):
                        pq = psq.tile([P, TCH], FP32, tag="pq")
                        for k in range(NK):
                            nc.tensor.matmul(
                                pq[:], wq_sb[k][:, m * P:(m + 1) * P], xT[k][:],
                                start=(k == 0), stop=(k == NK - 1),
                            )
                        qt = qhp.tile([P, TCH], BF16, tag=f"qh{m}", name=f"qh{m}")
                        nc.vector.tensor_copy(qt[:], pq[:])
                        qh.append(qt)

                    outT = [otp.tile([P, TCH], BF16, tag=f"ot{m}", name=f"ot{m}")
                            for m in range(NK)]
                    se_eo = [sep.tile([1, (H // 2) * TCH], BF16, tag=f"se{i}",
                                      name=f"se{i}") for i in range(2)]
                    for h in range(H):
                        m, off = h // 2, (h % 2) * 64
                        ex = []
                        for mc in range(2):
                            ps = pss.tile([P, TCH], FP32, tag=f"s{mc}")
                            nc.tensor.matmul(
                                ps[:],
                                kc_sb[m][off:off + 64, mc * P:(mc + 1) * P],
                                qh[m][off:off + 64, :],
                                start=True, stop=True,
                            )
                            e = expp.tile([P, TCH], BF16, tag="exp")
                            nc.scalar.activation(
                                e[:], ps[:], mybir.ActivationFunctionType.Exp,
                                bias=logc[:, mc:mc + 1], scale=0.125,
                            )
                            ex.append(e)
                        pav = psav.tile([65, TCH], FP32, tag="av")
                        for mc in range(2):
                            nc.tensor.matmul(
                                pav[:], vca[mc][:, h * 65:(h + 1) * 65], ex[mc][:],
                                start=(mc == 0), stop=(mc == 1),
                            )
                        se_dst = se_eo[h % 2][0:1, (h // 2) * TCH:
                                             (h // 2 + 1) * TCH]
                        if h % 2 == 0:
                            nc.vector.tensor_copy(se_dst, pav[64:65, :])
                            nc.scalar.copy(outT[m][off:off + 64, :], pav[0:64, :])
                        else:
                            nc.scalar.copy(se_dst, pav[64:65, :])
                            nc.vector.tensor_copy(
                                outT[m][off:off + 64, :], pav[0:64, :]
                            )

                    # batched 1/sumexp across all DVE lanes
                    sq = sep.tile([P, TCH // 8], FP32, tag="sq")
                    for i in range(2):
                        nc.gpsimd.dma_start(
                            out=sq[i * 64:(i + 1) * 64, :],
                            in_=se_eo[i].rearrange("a (p t) -> a p t", t=TCH),
                        )
                    rq = sep.tile([P, TCH // 8], FP32, tag="rq")
                    nc.vector.reciprocal(rq[:], sq[:])
                    rec_eo = [sep.tile([1, (H // 2) * TCH], BF16, tag=f"rec{i}",
                                       name=f"rec{i}") for i in range(2)]
                    for i in range(2):
                        nc.gpsimd.dma_start(
                            out=rec_eo[i].rearrange("a (p t) -> a p t", t=TCH),
                            in_=rq[i * 64:(i + 1) * 64, :],
                        )
                    for p in range(H // 2):
                        pbc = psf.tile([P, TCH], FP32, tag="pf")
                        for i in range(2):
                            nc.tensor.matmul(
                                pbc[i * 64:(i + 1) * 64, :], ones_row[:],
                                rec_eo[i][0:1, p * TCH:(p + 1) * TCH],
                                start=True, stop=True,
                            )
                        nc.vector.tensor_mul(outT[p][:], outT[p][:], pbc[:])

                    for mt in range(NMT):
                        pf = [psf.tile([P, 512], FP32, tag="pf", name=f"pf{nn}")
                              for nn in range(2)]
                        for k in range(NK):
                            for nn in range(2):
                                nc.tensor.matmul(
                                    pf[nn][:],
                                    outT[k][:, mt * P:(mt + 1) * P],
                                    wp_sb[k][:, nn * 512:(nn + 1) * 512],
                                    start=(k == 0), stop=(k == NK - 1),
                                )
                        for nn in range(2):
                            fin = finp.tile([P, 512], FP32, tag="fin")
                            nc.vector.tensor_add(
                                fin[:], pf[nn][:],
                                b_bc[:, nn * 512:(nn + 1) * 512]
                            )
                            nc.gpsimd.dma_start(
                                out=out_d[t0 + mt * P:t0 + (mt + 1) * P,
                                          nn * 512:(nn + 1) * 512],
                                in_=fin[:],
                            )
    nc.compile()
    return nc


_NC = None


def _get_nc():
    global _NC
    if _NC is None:
        _NC = build_nc()
    return _NC


def make_in_maps(cluster, q, w_q, w_kv, w_proj, b_proj):
    cluster = np.ascontiguousarray(np.asarray(cluster).astype(np.int32, copy=False))
    q = np.asarray(q, dtype=np.float32)
    w_q = np.ascontiguousarray(np.asarray(w_q, dtype=np.float32))
    w_kv = np.asarray(w_kv, dtype=np.float32)
    w_k = np.ascontiguousarray(w_kv[:, :D])
    w_v = np.ascontiguousarray(w_kv[:, D:])
    w_proj = np.ascontiguousarray(np.asarray(w_proj, dtype=np.float32))
    b_proj = np.ascontiguousarray(
        np.asarray(b_proj, dtype=np.float32).reshape(1, D)
    )
    return [
        {
            "x": np.ascontiguousarray(q[i]),
            "cluster": cluster[i],
            "w_q": w_q,
            "w_k": w_k,
            "w_v": w_v,
            "w_proj": w_proj,
            "b_proj": b_proj,
        }
        for i in range(q.shape[0])
    ]


def kernel(cluster, q, w_q, w_kv, w_proj, b_proj):
    global LAST_RESULTS
    from concourse.bass_utils import run_bass_kernel_spmd

    nc = _get_nc()
    in_maps = make_in_maps(cluster, q, w_q, w_kv, w_proj, b_proj)
    ncores = len(in_maps)
    res = run_bass_kernel_spmd(
        nc, in_maps, core_ids=list(range(ncores)), trace=TRACE
    )
    LAST_RESULTS = res
    return np.stack([res.results[i]["out"] for i in range(ncores)], axis=0)


# revision 15
# speedup vs baseline: 1.0583x; 1.0583x over previous
"""AdaptiveClusteringAttention TRN2 kernel (v4).

Data-parallel over batch: b=8 rows -> 8 NeuronCores, one row per core,
weights replicated. No collectives.

Per-core math (n=4096 tokens, d=1024, C=256 clusters, H=16 heads, dh=64):
  xc[c,:]  = sum_{t: cluster[t]=c} x[t,:]            (onehot matmul, raw sums)
  cnt[c]   = |{t: cluster[t]=c}|
  kc       = (xc / max(cnt,.5)) @ w_k                (mean k per cluster)
  vc'      = xc @ w_v                                (= cnt * v_center!)
  qh       = x @ w_q
  s[t,c]   = qh_h[t] . kc_h[c] / 8
  out      = (exp(s) @ vc'_h) / (exp(s) . cnt)       (count-weighted softmax)
  y        = out @ w_proj + b_proj

The count-weighting is folded into vc' (no log-count softmax bias), so exp
needs no per-partition bias and both cluster halves batch into one ACT call.
The per-token denominator comes free from a cnt column appended to vc'.

Precision: fp8 is confined to the score path (x^T, w_q, qh, kc) where
softmax sensitivity suppresses its ~2.8%/tensor quantization noise; the
value path stays bf16 (matmul operand quantization noise passes through to
the output at full strength). qh runs as fp8 DoubleRow during the phase-A
DMA stream; x^T comes from a bf16 DRAM round trip + XBAR DMA-transpose.

Phase C is software-pipelined: chunk ch-1's output projection and 1/sumexp
broadcast are interleaved into chunk ch's attention-head loop so the PE
stays dense (HAM stays at K=8/8) while ACT computes the exps.
"""

import os
import sys

import numpy as np

for _p in ("/opt/trn_rl_repo", os.path.expanduser("~/.axon_site/_ro/trn_rl_repo")):
    if os.path.isdir(_p) and _p not in sys.path:
        sys.path.append(_p)

import concourse.bass as bass  # noqa: E402
import concourse.mybir as mybir  # noqa: E402
import concourse.tile as tile  # noqa: E402
from concourse import bacc  # noqa: E402
from concourse.masks import make_identity  # noqa: E402

FP32 = mybir.dt.float32
BF16 = mybir.dt.bfloat16
F8 = mybir.dt.float8e4
I32 = mybir.dt.int32
DR = mybir.MatmulPerfMode.DoubleRow

N, D, C, H, DH, P = 4096, 1024, 256, 16, 64, 128
NJ = N // P          # 32 token row-tiles
NK = D // P          # 8 contraction chunks
TCH = 512            # token chunk for the attention phase
NCH = N // TCH       # 8 chunks
NMT = TCH // P       # 4 token subtiles per chunk

TRACE = False
LAST_RESULTS = None


def build_nc():
    nc = bacc.Bacc("TRN2", target_bir_lowering=False, debug=False)

    x_d = nc.dram_tensor("x", [N, D], FP32, kind="ExternalInput").ap()
    cl_d = nc.dram_tensor("cluster", [N], I32, kind="ExternalInput").ap()
    wq_d = nc.dram_tensor("w_q", [D, D], FP32, kind="ExternalInput").ap()
    wk_d = nc.dram_tensor("w_k", [D, D], FP32, kind="ExternalInput").ap()
    wv_d = nc.dram_tensor("w_v", [D, D], FP32, kind="ExternalInput").ap()
    wp_d = nc.dram_tensor("w_proj", [D, D], FP32, kind="ExternalInput").ap()
    bp_d = nc.dram_tensor("b_proj", [1, D], FP32, kind="ExternalInput").ap()
    out_d = nc.dram_tensor("out", [N, D], FP32, kind="ExternalOutput").ap()

    with tile.TileContext(nc) as tc:
        with (
            tc.tile_pool(name="dram", bufs=1, space="DRAM") as dram,
            tc.tile_pool(name="wts", bufs=1) as wts,
        ):
            xbf_d = dram.tile([N, D], BF16)

            # ---- constants ----
            iota_i = wts.tile([P, C], I32, tag="iota_i")
            nc.gpsimd.iota(iota_i[:], pattern=[[1, C]], base=0, channel_multiplier=0)
            iota_b = wts.tile([P, C], BF16, tag="iota_b")
            nc.vector.tensor_copy(iota_b[:], iota_i[:])
            ident32 = wts.tile([32, 32], BF16, tag="ident32")
            make_identity(nc, ident32[:])
            one11 = wts.tile([1, 1], BF16, tag="one11")
            nc.vector.memset(one11[:], 1.0)
            ones_col = wts.tile([P, 1], BF16, tag="ones_col")
            nc.vector.memset(ones_col[:], 1.0)
            ones_row = wts.tile([1, 64], BF16, tag="ones_row")
            nc.vector.memset(ones_row[:], 1.0)
            ones16 = wts.tile([P, 16], BF16, tag="ones16")
            nc.vector.memset(ones16[:], 1.0)
            ones16v = ones16.rearrange("p (h e) -> p h e", e=1)

            bp_sb = wts.tile([1, D], BF16, tag="bp_sb")
            nc.gpsimd.dma_start(out=bp_sb[:], in_=bp_d)
            b_bc = wts.tile([P, D], BF16, tag="b_bc")
            nc.gpsimd.partition_broadcast(b_bc[:], bp_sb[:])

            cl_i = wts.tile([NJ, P], I32, tag="cl_i")
            nc.sync.dma_start(out=cl_i[:], in_=cl_d.rearrange("(a b) -> a b", b=P))
            cl_b = wts.tile([NJ, P], BF16, tag="cl_b")
            nc.vector.tensor_copy(cl_b[:], cl_i[:])
            clusT = wts.tile([P, NJ], FP32, tag="clusT")
            with tc.tile_pool(name="psct", bufs=1, space="PSUM") as psct:
                ct_ps = psct.tile([P, NJ], BF16, tag="ct")
                nc.tensor.transpose(ct_ps[:], cl_b[:], ident32[:])
                nc.vector.tensor_copy(clusT[:], ct_ps[:])

            # ---- persistent data tiles ----
            wq8 = wts.tile([P, NK * D], F8, tag="wq8")
            wq8v = wq8.rearrange("p (k n) -> p k n", n=D)

            qh8 = [wts.tile([P, TCH], F8, tag=f"qh{i}", name=f"qh{i}")
                   for i in range(NCH * NK)]
            kc8 = [wts.tile([P, C], F8, tag=f"kc{m}", name=f"kc{m}")
                   for m in range(NK)]
            vca = [wts.tile([P, H * 65], BF16, tag=f"vca{i}", name=f"vca{i}")
                   for i in range(2)]
            xcm = [wts.tile([P, C], BF16, tag=f"xcm{m}", name=f"xcm{m}")
                   for m in range(NK)]
            xcr = [wts.tile([P, C], BF16, tag=f"xcr{m}", name=f"xcr{m}")
                   for m in range(NK)]

            cnt_sb = wts.tile([1, C], FP32, tag="cnt_sb")
            cnt_bf = wts.tile([1, C], BF16, tag="cnt_bf")
            cm_row = wts.tile([1, C], FP32, tag="cm_row")
            inv_row = wts.tile([1, C], FP32, tag="inv_row")
            inv_bc = wts.tile([P, C], FP32, tag="inv_bc")
            cnt_col = wts.tile([P, 2], FP32, tag="cnt_col")

            wk_sb = [wts.tile([P, D], BF16, tag=f"wk{k}", name=f"wk{k}")
                     for k in range(NK)]
            wv_sb = [wts.tile([P, D], BF16, tag=f"wv{k}", name=f"wv{k}")
                     for k in range(NK)]

            # ---- phase A: stream x, counts, x^T round trip, qh, xc ----
            with (
                tc.tile_pool(name="xin", bufs=1) as xin,
                tc.tile_pool(name="ohp", bufs=1) as ohp,
                tc.tile_pool(name="xtb", bufs=2) as xtb,
                tc.tile_pool(name="xtp8", bufs=2) as xtp8,
                tc.tile_pool(name="wst", bufs=2) as wst,
                tc.tile_pool(name="psA", bufs=1, space="PSUM") as psA,
                tc.tile_pool(name="psAm", bufs=3, space="PSUM") as psAm,
                tc.tile_pool(name="psT", bufs=1, space="PSUM") as psT,
                tc.tile_pool(name="psq", bufs=2, space="PSUM") as psq,
            ):
                # w_q first (qh needs it immediately): bf16 cast-DMA -> fp8
                for k in range(NK):
                    st = wst.tile([P, D], BF16, tag="wst")
                    nc.gpsimd.dma_start(out=st[:], in_=wq_d[k * P:(k + 1) * P, :])
                    nc.scalar.copy(wq8v[:, k:k + 1, :], st[:])

                pcnt = psA.tile([1, C], FP32, tag="cnt")
                xall, ohall = [], []
                for ch in range(NCH):
                    for jj in range(NMT):
                        j = ch * NMT + jj
                        xj = xin.tile([P, D], BF16, tag=f"xj{j}", name=f"xj{j}")
                        nc.gpsimd.dma_start(out=xj[:], in_=x_d[j * P:(j + 1) * P, :])
                        nc.sync.dma_start(out=xbf_d[j * P:(j + 1) * P, :],
                                          in_=xj[:])
                        oh = ohp.tile([P, C], BF16, tag=f"oh{j}", name=f"oh{j}")
                        nc.vector.tensor_scalar(
                            oh[:], iota_b[:], clusT[:, j:j + 1], None,
                            mybir.AluOpType.is_equal,
                        )
                        nc.tensor.matmul(pcnt[:], ones_col[:], oh[:],
                                         start=(j == 0), stop=(j == NJ - 1))
                        xall.append(xj)
                        ohall.append(oh)
                    # trickle k/v weights in (bf16 cast-DMA, one slab each/chunk)
                    nc.gpsimd.dma_start(out=wk_sb[ch][:],
                                        in_=wk_d[ch * P:(ch + 1) * P, :])
                    nc.gpsimd.dma_start(out=wv_sb[ch][:],
                                        in_=wv_d[ch * P:(ch + 1) * P, :])
                    # x^T for this chunk: XBAR DMA-transpose + one fp8 cast
                    t0 = ch * TCH
                    xt = xtb.tile([P, NK * TCH], BF16, tag="xt")
                    for k in range(NK):
                        nc.sync.dma_start_transpose(
                            out=xt[:, k * TCH:(k + 1) * TCH],
                            in_=xbf_d[t0:t0 + TCH, k * P:(k + 1) * P],
                        )
                    xt8 = xtp8.tile([P, NK * TCH], F8, tag="xt8")
                    nc.vector.tensor_copy(xt8[:], xt[:])
                    xt8v = xt8.rearrange("p (k t) -> p k t", t=TCH)
                    # qh^T for this chunk (fp8 DoubleRow over d)
                    for m in range(NK):
                        pq = psq.tile([P, TCH], FP32, tag="pq")
                        for j2 in range(NK // 2):
                            nc.tensor.matmul(
                                pq[:],
                                wq8v[:, 2 * j2:2 * j2 + 2, m * P:(m + 1) * P],
                                xt8v[:, 2 * j2:2 * j2 + 2, :],
                                start=(j2 == 0), stop=(j2 == NK // 2 - 1),
                                perf_mode=DR,
                            )
                        nc.scalar.copy(qh8[ch * NK + m][:], pq[:])

                # counts -> inv row (for k means) + raw column (for vc')
                nc.scalar.copy(cnt_sb[:], pcnt[:])
                nc.vector.tensor_copy(cnt_bf[:], cnt_sb[:])
                nc.vector.tensor_scalar_max(cm_row[:], cnt_sb[:], 0.5)
                nc.vector.reciprocal(inv_row[:], cm_row[:])
                nc.gpsimd.partition_broadcast(inv_bc[:], inv_row[:])
                for mc in range(2):
                    pt = psT.tile([P, 1], BF16, tag="pt")
                    nc.tensor.matmul(
                        pt[:], cnt_bf[0:1, mc * P:(mc + 1) * P], one11[:],
                        is_transpose=True,
                    )
                    nc.scalar.copy(cnt_col[:, mc:mc + 1], pt[:])

                # xc^T (raw cluster sums, d-major) and mean version for kc
                for m in range(NK):
                    pxc = psAm.tile([P, C], FP32, tag="pxc")
                    for j in range(NJ):
                        nc.tensor.matmul(
                            pxc[:], xall[j][:, m * P:(m + 1) * P], ohall[j][:],
                            start=(j == 0), stop=(j == NJ - 1),
                        )
                    nc.vector.tensor_mul(xcm[m][:], pxc[:], inv_bc[:])
                    nc.scalar.copy(xcr[m][:], pxc[:])

            # w_proj tiles live only from phase B on (SBUF headroom in A)
            with tc.tile_pool(name="wpp", bufs=1) as wpp:
                wp_sb = [wpp.tile([P, D], BF16, tag=f"wp{k}", name=f"wp{k}")
                         for k in range(NK)]
                for k in range(NK):
                    nc.gpsimd.dma_start(out=wp_sb[k][:],
                                        in_=wp_d[k * P:(k + 1) * P, :])

                # ---- phase B: kc^T, vc' = xc @ w_v (with cnt column) ----
                with (
                    tc.tile_pool(name="psBk", bufs=2, space="PSUM") as psBk,
                    tc.tile_pool(name="psBv", bufs=2, space="PSUM") as psBv,
                ):
                    for m in range(NK):
                        pk = psBk.tile([P, C], FP32, tag="pk")
                        for k in range(NK):
                            nc.tensor.matmul(
                                pk[:], wk_sb[k][:, m * P:(m + 1) * P], xcm[k][:],
                                start=(k == 0), stop=(k == NK - 1),
                            )
                        nc.vector.tensor_copy(kc8[m][:], pk[:])
                    for mc in range(2):
                        va = vca[mc].rearrange("p (h e) -> p h e", e=65)
                        nc.vector.tensor_scalar(
                            va[:, :, 64:65], ones16v[:], cnt_col[:, mc:mc + 1],
                            None, mybir.AluOpType.mult,
                        )
                        for nn in range(2):
                            pv = psBv.tile([P, 512], FP32, tag="pv")
                            for k in range(NK):
                                nc.tensor.matmul(
                                    pv[:], xcr[k][:, mc * P:(mc + 1) * P],
                                    wv_sb[k][:, nn * 512:(nn + 1) * 512],
                                    start=(k == 0), stop=(k == NK - 1),
                                )
                            nc.vector.tensor_copy(
                                va[:, nn * 8:(nn + 1) * 8, 0:64],
                                pv.rearrange("p (h e) -> p h e", e=64),
                            )

                # ---- phase C: software-pipelined attention + output proj ----
                with (
                    tc.tile_pool(name="exq", bufs=3) as exq,
                    tc.tile_pool(name="otp", bufs=2) as otp,
                    tc.tile_pool(name="sep", bufs=2) as sep,
                    tc.tile_pool(name="finp", bufs=3) as finp,
                    tc.tile_pool(name="pss", bufs=2, space="PSUM") as pss,
                    tc.tile_pool(name="psav", bufs=2, space="PSUM") as psav,
                    tc.tile_pool(name="psf", bufs=2, space="PSUM") as psf,
                ):
                    def emit_scores_pair(ch, hp, st):
                        # two heads, alternating PE row-groups (0-63 / 64-127)
                        # so consecutive matmuls overlap in the array
                        s2s = [pss.tile([P, 2 * TCH], FP32, tag="s", name=f"s{i}")
                               for i in range(2)]
                        for mc in range(2):
                            for par in range(2):
                                s3 = s2s[par].rearrange("p (m t) -> p m t", t=TCH)
                                nc.tensor.matmul(
                                    s3[:, mc:mc + 1, :],
                                    kc8[hp][par * 64:par * 64 + 64,
                                            mc * P:(mc + 1) * P],
                                    qh8[ch * NK + hp][par * 64:par * 64 + 64, :],
                                    start=True, stop=True,
                                )
                        for par in range(2):
                            ex2 = exq.tile([P, 2 * TCH], BF16, tag="ex")
                            nc.scalar.activation(
                                ex2[:], s2s[par][:],
                                mybir.ActivationFunctionType.Exp, scale=0.125,
                            )
                            st["ex"].append(ex2)

                    def emit_av_head(ch, h, st):
                        m, off = h // 2, (h % 2) * 64
                        ex3 = st["ex"][h].rearrange("p (m t) -> p m t", t=TCH)
                        pav = psav.tile([65, TCH], FP32, tag="av")
                        for mc in range(2):
                            nc.tensor.matmul(
                                pav[:], vca[mc][:, h * 65:(h + 1) * 65],
                                ex3[:, mc:mc + 1, :],
                                start=(mc == 0), stop=(mc == 1),
                            )
                        se_dst = st["se"][0:1, h * TCH:(h + 1) * TCH]
                        nc.vector.tensor_copy(se_dst, pav[64:65, :])
                        if h % 2 == 0:
                            nc.scalar.copy(st["outT"][m][off:off + 64, :],
                                           pav[0:64, :])
                        else:
                            nc.vector.tensor_copy(st["outT"][m][off:off + 64, :],
                                                  pav[0:64, :])

                    def emit_recip(st):
                        # 1/sumexp per (head, token) via one SBUF round trip
                        sq = sep.tile([P, H * TCH // P], FP32, tag="sq")
                        nc.gpsimd.dma_start(
                            out=sq[:],
                            in_=st["se"].rearrange("a (p t) -> a p t", t=TCH),
                        )
                        rq = sep.tile([P, H * TCH // P], FP32, tag="rq")
                        nc.vector.reciprocal(rq[:], sq[:])
                        rec = sep.tile([1, H * TCH], BF16, tag="rec")
                        nc.gpsimd.dma_start(
                            out=rec.rearrange("a (p t) -> a p t", t=TCH),
                            in_=rq[:],
                        )
                        st["rec"] = rec

                    def emit_pbc_muls(st):
                        rec = st["rec"]
                        for m in range(NK):
                            pbc = psf.tile([P, TCH], FP32, tag="pf")
                            for par in range(2):
                                h = 2 * m + par
                                nc.tensor.matmul(
                                    pbc[par * 64:(par + 1) * 64, :], ones_row[:],
                                    rec[0:1, h * TCH:(h + 1) * TCH],
                                    start=True, stop=True,
                                )
                            nc.vector.tensor_mul(st["outT"][m][:],
                                                 st["outT"][m][:], pbc[:])

                    def emit_proj_mt(st, mt):
                        t0 = st["ch"] * TCH
                        pf = [psf.tile([P, 512], FP32, tag="pf", name=f"pf{nn}")
                              for nn in range(2)]
                        for k in range(NK):
                            for nn in range(2):
                                nc.tensor.matmul(
                                    pf[nn][:],
                                    st["outT"][k][:, mt * P:(mt + 1) * P],
                                    wp_sb[k][:, nn * 512:(nn + 1) * 512],
                                    start=(k == 0), stop=(k == NK - 1),
                                )
                        for nn in range(2):
                            fin = finp.tile([P, 512], FP32, tag="fin")
                            nc.vector.tensor_add(
                                fin[:], pf[nn][:],
                                b_bc[:, nn * 512:(nn + 1) * 512]
                            )
                            nc.sync.dma_start(
                                out=out_d[t0 + mt * P:t0 + (mt + 1) * P,
                                          nn * 512:(nn + 1) * 512],
                                in_=fin[:],
                            )

                    prev = None
                    for ch in range(NCH + 1):
                        if ch < NCH:
                            st = {
                                "ch": ch,
                                "ex": [],
                                "outT": [otp.tile([P, TCH], BF16, tag=f"ot{m}",
                                                  name=f"ot{m}")
                                         for m in range(NK)],
                                "se": sep.tile([1, H * TCH], BF16, tag="se",
                                               name="se"),
                            }
                            for hp in range(H // 2):
                                # prev chunk's deferred work first (no deps on
                                # this chunk) to keep the PE stream dense
                                if prev is not None:
                                    if hp == 1:
                                        emit_pbc_muls(prev)
                                    elif hp in (2, 4, 5, 6):
                                        mt = {2: 0, 4: 1, 5: 2, 6: 3}[hp]
                                        emit_proj_mt(prev, mt)
                                emit_scores_pair(ch, hp, st)
                                if hp > 0:
                                    emit_av_head(ch, 2 * hp - 2, st)
                                    emit_av_head(ch, 2 * hp - 1, st)
                            emit_av_head(ch, H - 2, st)
                            emit_av_head(ch, H - 1, st)
                            emit_recip(st)
                            prev = st
                        else:
                            emit_pbc_muls(prev)
                            for mt in range(NMT):
                                emit_proj_mt(prev, mt)
    nc.compile()
    return nc


_NC = None


def _get_nc():
    global _NC
    if _NC is None:
        _NC = build_nc()
    return _NC


def make_in_maps(cluster, q, w_q, w_kv, w_proj, b_proj):
    cluster = np.ascontiguousarray(np.asarray(cluster).astype(np.int32, copy=False))
    q = np.asarray(q, dtype=np.float32)
    w_q = np.ascontiguousarray(np.asarray(w_q, dtype=np.float32))
    w_kv = np.asarray(w_kv, dtype=np.float32)
    w_k = np.ascontiguousarray(w_kv[:, :D])
    w_v = np.ascontiguousarray(w_kv[:, D:])
    w_proj = np.ascontiguousarray(np.asarray(w_proj, dtype=np.float32))
    b_proj = np.ascontiguousarray(
        np.asarray(b_proj, dtype=np.float32).reshape(1, D)
    )
    return [
        {
            "x": np.ascontiguousarray(q[i]),
            "cluster": cluster[i],
            "w_q": w_q,
            "w_k": w_k,
            "w_v": w_v,
            "w_proj": w_proj,
            "b_proj": b_proj,
        }
        for i in range(q.shape[0])
    ]


def kernel(cluster, q, w_q, w_kv, w_proj, b_proj):
    global LAST_RESULTS
    from concourse.bass_utils import run_bass_kernel_spmd

    nc = _get_nc()
    in_maps = make_in_maps(cluster, q, w_q, w_kv, w_proj, b_proj)
    ncores = len(in_maps)
    res = run_bass_kernel_spmd(
        nc, in_maps, core_ids=list(range(ncores)), trace=TRACE
    )
    LAST_RESULTS = res
    return np.stack([res.results[i]["out"] for i in range(ncores)], axis=0)
